# revision 47
# baseline (speedup 1.0000x reference)
"""Self-contained LSTM encoder/decoder kernel for TRN2 (8-core SPMD).

Per-core (data-parallel over batch N=512 -> B=64), the batch is split
into 2 independent chains of BC=32 that are software-pipelined across
the engines: per chain-step
  PE 4 gate matmuls -> ACT tanh(ifgo, one instr) -> DVE uv ->
  DVE C2' -> ACT tanh(c) -> DVE h'.

Math tricks (folded into host-side weight preprocessing):
  - sigma(x) = (tanh(x/2) + 1) / 2  -> ONE tanh over all 4 gates
  - state stored doubled: H2 = 2h, C2 = 2c
      u  = (yi + 1) * yg; v = (yf + 1) * C2
      C2'= 0.5*v + u;  tc = tanh(0.5*C2');  H2' = (yo + 1) * tc
  - decoder feedback folded: gates_{t+1} = h_t @ (fcW.T@dec_Wih.T
    + dec_Whh.T) + bias
  - biases via a ones-row in the h operand (K=101)
  - gate weight chunks padded to 128 columns (keeps LDWEIGHTS 32-col
    aligned; 100-col chunks measured 2x slower)
  - encoder x-projections batched per 8 steps in PSUM group tiles,
    split per-chain so the two chains never share a PSUM bank
  - a sim-time gate (tile_wait_until) paces the Tile scheduler so the
    baked per-engine order interleaves the two chains

Scheduling tricks (each verified on HW):
  - gate-psum tiles are pre-cleared by a dummy matmul with zero
    weights (start=True) so the first real gate MM carries ONLY the
    h2 RAW wait; a second wait would be migrated onto its LDWEIGHTS
    by move_matmul_waits_to_ldweights and stall the weight prefetch
    until h2 is ready (~150ns/step on the recurrence critical path)
  - the decoder FC output COPY (PSUM->SBUF) + DMAs are deferred to a
    late scheduling gate (base+1.25 phases) so the DVE FIFO does not
    head-of-line block the next tail's h2 stt (~190ns/step); the FC
    matmul itself stays early (the next h2 write has a WAR on it)
"""

import numpy as np

import concourse.bass as bass
import concourse.tile as tile
from concourse import bacc, mybir

F32 = mybir.dt.float32
F16 = mybir.dt.float16
AF = mybir.ActivationFunctionType
ALU = mybir.AluOpType

B = 64          # batch per core
BC = 32         # batch per chain
H = 100
F = 150
G = 4 * 100
GW = 128        # weight columns per gate chunk. Padding to 128 keeps the
                # LDWEIGHTS 32-col-group aligned (100-col chunks measured
                # 2x slower: 203ns vs 105ns per load)
GP = 4 * GW
KA = 128
KB = F - KA
T = 300
N_CORES = 8


ENC_PRECLEAR = True
DEC_PRECLEAR = True
H2_ON_GPSIMD = False


def build_nc2(T=300, gates_group=8, gate_ns=940):
    D = T - 1
    nc = bacc.Bacc("TRN2", target_bir_lowering=False, debug=False,
                   enable_asserts=False)

    xa_d = nc.dram_tensor("xa", [KA, T, B], F16, kind="ExternalInput")
    xb_d = nc.dram_tensor("xb", [KB, T, B], F16, kind="ExternalInput")
    w_iha_d = nc.dram_tensor("w_iha", [KA, GP], F16, kind="ExternalInput")
    w_ihb_d = nc.dram_tensor("w_ihb", [KB, GP], F16, kind="ExternalInput")
    w_hh_d = nc.dram_tensor("w_hh", [H + 1, GP], F16, kind="ExternalInput")
    w_d0_d = nc.dram_tensor("w_d0", [H + 1, GP], F16, kind="ExternalInput")
    w_dc_d = nc.dram_tensor("w_dc", [H + 1, GP], F16, kind="ExternalInput")
    w_fc_d = nc.dram_tensor("w_fc", [H + 1, F], F16, kind="ExternalInput")
    out_d = nc.dram_tensor("out", [D, 2, BC, F], F32, kind="ExternalOutput")

    GS = gates_group
    CHUNK_GROUPS = 4
    CHUNK = GS * B * CHUNK_GROUPS

    with tile.TileContext(nc) as tc:
        with (
            tc.tile_pool(name="const", bufs=1) as constp,
            tc.tile_pool(name="state", bufs=1) as statep,
            tc.tile_pool(name="xchunk", bufs=2) as xpool,
            tc.tile_pool(name="ypool", bufs=12) as ypool,
            tc.tile_pool(name="work", bufs=12) as work,
        ):
            w_iha = constp.tile([KA, GP], F16, tag="w_iha")
            w_ihb = constp.tile([KB, GP], F16, tag="w_ihb")
            w_hh = constp.tile([H + 1, GP], F16, tag="w_hh")
            w_d0 = constp.tile([H + 1, GP], F16, tag="w_d0")
            w_dc = constp.tile([H + 1, GP], F16, tag="w_dc")
            w_fc = constp.tile([H + 1, F], F16, tag="w_fc")
            # Zero weights: dummy matmuls pre-clear each gate-psum tile so
            # the real first gate MM carries only the h2 RAW wait (a 2nd
            # wait would migrate onto its LDWEIGHTS and stall the weight
            # prefetch until h2 is ready).
            zw = constp.tile([H + 1, GW], F16, tag="zw")
            # Encoder weights first, spread across engine DGE queues so the
            # startup transfers run in parallel instead of serializing on
            # the Sync queue; decoder weights (w_d0/w_dc/w_fc, not needed
            # for ~670us) are DMA'd after the first x chunk.
            nc.scalar.dma_start(out=w_iha[:], in_=w_iha_d.ap())
            nc.scalar.dma_start(out=w_ihb[:], in_=w_ihb_d.ap())
            nc.gpsimd.dma_start(out=w_hh[:], in_=w_hh_d.ap())

            nc.vector.memset(zw[:], 0.0)

            # h2[part, chain, parity*BC + col]; h_k lives at parity k%2.
            h2 = statep.tile([H + 1, 2, 2 * BC], F16, tag="h2")
            nc.vector.memset(h2[0:96, :, :], 0.0)
            nc.vector.memset(h2[96:H + 1, :, :], 1.0)
            nc.vector.memset(h2[96:H, :, :], 0.0)

            y_cur = [None, None]
            y_nxt = [None, None]
            tc_t = [None, None]
            for X in (0, 1):
                y_cur[X] = ypool.tile([H, 5, BC], F16, tag=f"y{X}",
                                      name=f"y{X}")
                nc.vector.memset(y_cur[X][:, 4, :], 0.0)

            def tg(X, ap_gates):
                nc.scalar.activation(y_cur[X][:, 0:4, :], ap_gates, AF.Tanh)

            def uv_cc(X):
                uv = work.tile([H, 2, BC], F16, tag=f"uv{X}", name=f"uv{X}")
                nc.vector.scalar_tensor_tensor(
                    uv[:], y_cur[X][:, 0:2, :], 1.0, y_cur[X][:, 2:5:2, :],
                    ALU.add, ALU.mult)
                y_nxt[X] = ypool.tile([H, 5, BC], F16, tag=f"y{X}",
                                      name=f"y{X}")
                nc.vector.scalar_tensor_tensor(
                    y_nxt[X][:, 4, :], uv[:, 1, :], 0.5, uv[:, 0, :],
                    ALU.mult, ALU.add)

            h2_eng = nc.gpsimd if H2_ON_GPSIMD else nc.vector

            def tail(X, wp):
                """TC + H2 for chain X's completed step -> h2 parity wp."""
                tc_t[X] = work.tile([H, BC], F16, tag=f"tc{X}",
                                    name=f"tcx{X}")
                nc.scalar.activation(tc_t[X][:], y_nxt[X][:, 4, :], AF.Tanh,
                                     scale=0.5)
                h2_eng.scalar_tensor_tensor(
                    h2[0:H, X, wp * BC:(wp + 1) * BC],
                    y_cur[X][:, 3, :], 1.0, tc_t[X][:],
                    ALU.add, ALU.mult)
                y_cur[X] = y_nxt[X]

            phase_idx = [0]

            def next_base():
                k = phase_idx[0]
                phase_idx[0] += 1
                return k * gate_ns * 1e-6

            def sub(base, frac):
                return tc.tile_wait_until(base + frac * gate_ns * 1e-6)

            # =========== ENCODER ===========
            n_groups = (T + GS - 1) // GS
            CH_STEPS = GS * CHUNK_GROUPS
            with tc.tile_pool(name="epsum", bufs=2, space="PSUM") as epsum:
                chunks = {}

                def load_chunk(ci):
                    s0 = ci * CH_STEPS
                    s1 = min(T, s0 + CH_STEPS)
                    xa_sb = xpool.tile([KA, CH_STEPS, B], F16, tag="xa")
                    xb_sb = xpool.tile([KB, CH_STEPS, B], F16, tag="xb")
                    if ci == 0:
                        # Per-group pieces on alternating DGE queues:
                        # group 0's x-proj matmuls only depend on the
                        # first piece, and pieces transfer in parallel.
                        engs = [nc.sync, nc.gpsimd, nc.scalar]
                        for pi, p0 in enumerate(range(0, s1 - s0, GS)):
                            p1 = min(s1 - s0, p0 + GS)
                            eng = engs[pi % len(engs)]
                            eng.dma_start(
                                out=xa_sb[:, p0:p1, :],
                                in_=xa_d.ap()[:, s0 + p0:s0 + p1, :])
                            eng.dma_start(
                                out=xb_sb[:, p0:p1, :],
                                in_=xb_d.ap()[:, s0 + p0:s0 + p1, :])
                    else:
                        nc.sync.dma_start(out=xa_sb[:, 0:s1 - s0, :],
                                          in_=xa_d.ap()[:, s0:s1, :])
                        nc.sync.dma_start(out=xb_sb[:, 0:s1 - s0, :],
                                          in_=xb_d.ap()[:, s0:s1, :])
                    chunks[ci] = (xa_sb, xb_sb)

                def make_xmm_emitters(g8, pe4):
                    steps = min(GS, T - GS * g8)
                    nc2 = steps * BC
                    ci = g8 // CHUNK_GROUPS
                    offs = (g8 % CHUNK_GROUPS) * GS
                    ems = []
                    if ENC_PRECLEAR:
                        for X in (0, 1):
                            for gi in range(4):
                                def emz(X=X, gi=gi):
                                    nc.tensor.matmul(
                                        pe4[:, X, gi, 0:nc2], zw[:],
                                        w_hh[:, 0:nc2],
                                        start=True, stop=False)
                                ems.append(emz)
                    for X in (0, 1):
                        for gi in range(4):
                            def ema(X=X, gi=gi):
                                xa_sb, _ = chunks[ci]
                                nc.tensor.matmul(
                                    pe4[:, X, gi, 0:nc2],
                                    w_iha[:, gi * GW:(gi + 1) * GW],
                                    xa_sb[:, offs:offs + steps,
                                          X * BC:(X + 1) * BC],
                                    start=not ENC_PRECLEAR, stop=False)
                            def emb(X=X, gi=gi):
                                _, xb_sb = chunks[ci]
                                nc.tensor.matmul(
                                    pe4[:, X, gi, 0:nc2],
                                    w_ihb[:, gi * GW:(gi + 1) * GW],
                                    xb_sb[:, offs:offs + steps,
                                          X * BC:(X + 1) * BC],
                                    start=False, stop=False)
                            ems += [ema, emb]
                    return ems

                load_chunk(0)
                # Decoder weights: issue after the first x chunk so they
                # don't sit ahead of it on the Sync DMA queue.
                nc.sync.dma_start(out=w_d0[:], in_=w_d0_d.ap())
                nc.sync.dma_start(out=w_dc[:], in_=w_dc_d.ap())
                nc.sync.dma_start(out=w_fc[:], in_=w_fc_d.ap())
                pe_cur = epsum.tile([128, 2, 4, GS * BC], F32, tag="pe4")
                # PE warm-up bridge: keep the PE busy from ~7us (zw memset
                # done) until just before the first x piece lands (~15us) so
                # the HAM clock gate stays at 2.4GHz for the first real
                # x-projection matmuls (measured 417ns cold vs ~210 warm).
                # A shorter burst re-throttles: the MID window re-arms after
                # ~3.4us of PE idle. Output region is overwritten by the
                # real preclears below.
                for _ in range(60):
                    nc.tensor.matmul(pe_cur[:, 0, 0, 0:GW], zw[:],
                                     zw[:, 0:GW], start=True, stop=False,
                                     skip_group_check=True)
                for em in make_xmm_emitters(0, pe_cur):
                    em()
                pe_next, next_ems = None, []

                for g8 in range(n_groups):
                    steps = min(GS, T - GS * g8)
                    pci = (g8 + 2) // CHUNK_GROUPS
                    if pci * CHUNK < T * B and pci not in chunks:
                        load_chunk(pci)
                    if g8 + 1 < n_groups:
                        nci = (g8 + 1) // CHUNK_GROUPS
                        if nci not in chunks:
                            load_chunk(nci)
                        pe_next = epsum.tile([128, 2, 4, GS * BC], F32,
                                             tag="pe4")
                        next_ems = make_xmm_emitters(g8 + 1, pe_next)
                    else:
                        pe_next, next_ems = None, []
                    n_ph = 2 * steps
                    per_ph = -(-len(next_ems) // n_ph) if next_ems else 0
                    for s in range(steps):
                        t = GS * g8 + s
                        rp = t % 2
                        for X in (0, 1):
                            Y = 1 - X
                            base = next_base()
                            yt = t - 1 + X
                            with sub(base, 0.0):
                                if yt >= 0:
                                    tail(Y, (yt + 1) % 2)
                                col0 = s * BC
                                for gi in range(4):
                                    nc.tensor.matmul(
                                        pe_cur[:, X, gi, col0:col0 + BC],
                                        w_hh[:, gi * GW:(gi + 1) * GW],
                                        h2[:, X, rp * BC:(rp + 1) * BC],
                                        start=False, stop=True,
                                        skip_group_check=not (s == 0
                                                             and X == 0))
                                ph = 2 * s + X
                                for em in next_ems[ph * per_ph:
                                                   (ph + 1) * per_ph]:
                                    em()
                                tg(X, pe_cur[0:H, X, 0:4, col0:col0 + BC])
                                uv_cc(X)
                    pe_cur = pe_next
                # drain chain 1's step T-1 tail
                tail(1, T % 2)

            # =========== DECODER ===========
            with (
                tc.tile_pool(name="dpsum", bufs=3, space="PSUM") as dpsum,
                tc.tile_pool(name="fpsum", bufs=2, space="PSUM") as fpsum,
            ):
                PO = T % 2

                def emit_fc_mm(X):
                    """FC matmul for chain X (both h2 parities). Must stay
                    early: the next tail's h2 write has a WAR edge on it."""
                    pfc = fpsum.tile([2 * BC, F], F32, tag="pfc")
                    nc.tensor.matmul(pfc[:], h2[:, X, :], w_fc[:],
                                     start=True, stop=True)
                    return pfc

                def emit_fc_out(X, dp, pfc):
                    """PSUM->SBUF copy + output DMAs for steps (dp-1, dp).
                    Deferred to a late scheduling gate so the DVE COPY does
                    not head-of-line block the next tail's h2 stt."""
                    p1 = (dp + PO) % 2
                    ofc = work.tile([2 * BC, F], F32, tag=f"ofc{X}",
                                    name=f"ofc{X}")
                    nc.vector.tensor_copy(ofc[:], pfc[:])
                    nc.sync.dma_start(out=out_d.ap()[dp - 1, X],
                                      in_=ofc[p1 * BC:(p1 + 1) * BC, :])
                    nc.sync.dma_start(out=out_d.ap()[dp - 2, X],
                                      in_=ofc[(1 - p1) * BC:(2 - p1) * BC, :])

                for d in range(1, D + 1):
                    wd = w_d0 if d == 1 else w_dc
                    rp = (d - 1 + PO) % 2
                    for X in (0, 1):
                        Y = 1 - X
                        base = next_base()
                        yt = d - 1 + X
                        pfc = None
                        with sub(base, 0.0):
                            if (X == 1) or (d >= 2):
                                tail(Y, (yt + PO) % 2)
                            pd = dpsum.tile([128, 4 * BC], F32,
                                            tag=f"pd{X}", name=f"pd{X}")
                            if DEC_PRECLEAR:
                                nc.tensor.matmul(
                                    pd[:, :], zw[:], w_hh[:, 0:4 * BC],
                                    start=True, stop=False)
                            for gi in range(4):
                                nc.tensor.matmul(
                                    pd[:, gi * BC:(gi + 1) * BC],
                                    wd[:, gi * GW:(gi + 1) * GW],
                                    h2[:, X, rp * BC:(rp + 1) * BC],
                                    start=(gi == 0 and not DEC_PRECLEAR),
                                    stop=gi == 3)
                            if d >= 3 and d % 2 == 1:
                                pfc = emit_fc_mm(X)
                            tg(X, pd[0:H, :])
                            uv_cc(X)
                        if pfc is not None:
                            with sub(base, 1.25):
                                emit_fc_out(X, d - 1, pfc)
                # drain chain 1's step D tail
                tail(1, (D + PO) % 2)
                for X in (0, 1):
                    if D % 2 == 0:
                        pfc = emit_fc_mm(X)
                        emit_fc_out(X, D, pfc)
                    else:
                        pD = (D + PO) % 2
                        pfc = fpsum.tile([2 * BC, F], F32, tag="pfc")
                        nc.tensor.matmul(pfc[0:BC, :],
                                         h2[:, X, pD * BC:(pD + 1) * BC],
                                         w_fc[:], start=True, stop=True)
                        ofc = work.tile([2 * BC, F], F32, tag=f"ofc{X}",
                                        name=f"ofc{X}")
                        nc.vector.tensor_copy(ofc[0:BC, :], pfc[0:BC, :])
                        nc.sync.dma_start(out=out_d.ap()[D - 1, X],
                                          in_=ofc[0:BC, :])

    nc.compile()
    return nc


# ======================= host pre/post =======================

def _colscale():
    s = np.ones(G, np.float32)
    s[0:100] = 0.5    # i
    s[100:200] = 0.5  # f
    s[300:400] = 0.5  # o
    return s


def _pad_gates(w):
    K = w.shape[0]
    out = np.zeros((K, GP), w.dtype)
    for gi in range(4):
        out[:, gi * GW:gi * GW + 100] = w[:, gi * 100:(gi + 1) * 100]
    return out


def make_weight_arrays(enc_Wih, enc_Whh, enc_bih, enc_bhh,
                       dec_Wih, dec_Whh, dec_bih, dec_bhh, fc_W, fc_b):
    cs = _colscale()
    f64 = np.float64
    w_ih = (enc_Wih.T.astype(f64) * cs).astype(np.float32)
    w_hh = np.vstack([enc_Whh.T.astype(f64) * 0.5,
                      (enc_bih + enc_bhh).astype(f64)[None, :]]) * cs
    w_d0 = np.vstack([dec_Whh.T.astype(f64) * 0.5,
                      (dec_bih + dec_bhh).astype(f64)[None, :]]) * cs
    combo = fc_W.T.astype(f64) @ dec_Wih.T.astype(f64) + dec_Whh.T.astype(f64)
    bias_c = (fc_b.astype(f64) @ dec_Wih.T.astype(f64)
              + dec_bih.astype(f64) + dec_bhh.astype(f64))
    w_dc = np.vstack([combo * 0.5, bias_c[None, :]]) * cs
    w_fc = np.vstack([fc_W.T.astype(f64) * 0.5, fc_b.astype(f64)[None, :]])
    return {
        "w_iha": _pad_gates(w_ih[0:KA]).astype(np.float16),
        "w_ihb": _pad_gates(w_ih[KA:F]).astype(np.float16),
        "w_hh": _pad_gates(w_hh).astype(np.float16),
        "w_d0": _pad_gates(w_d0).astype(np.float16),
        "w_dc": _pad_gates(w_dc).astype(np.float16),
        "w_fc": w_fc.astype(np.float16),
    }


def make_x_arrays(x_shard, T):
    """x_shard (B, 3, T, 25, 2) -> xa (128, T, B), xb (22, T, B) fp16."""
    xt = np.ascontiguousarray(
        x_shard.transpose(1, 3, 4, 2, 0)).reshape(F, T, B)
    xt = xt.astype(np.float16)
    return {"xa": np.ascontiguousarray(xt[0:KA]),
            "xb": np.ascontiguousarray(xt[KA:F])}


def postprocess2(core_outs, T, n_cores=8):
    """core_outs: list of (D, 2, BC, F) -> full (N, 3, T, 25, 2)."""
    N = n_cores * B
    full = np.zeros((N, 3, T, 25, 2), np.float32)
    for i, o in enumerate(core_outs):
        ob = o.reshape(T - 1, B, 3, 25, 2).transpose(1, 2, 0, 3, 4)
        full[i * B:(i + 1) * B, :, 1:T] = ob
    return full


# ======================= self-contained kernel entry =======================

_NC_CACHE = {}


def _get_nc():
    if "nc" not in _NC_CACHE:
        _NC_CACHE["nc"] = build_nc2(T=T)
    return _NC_CACHE["nc"]


def kernel(x, enc_Wih, enc_Whh, enc_bih, enc_bhh,
           dec_Wih, dec_Whh, dec_bih, dec_bhh, fc_W, fc_b):
    from concourse.bass_utils import run_bass_kernel_spmd

    x = np.asarray(x, np.float32)
    nc = _get_nc()
    weights = make_weight_arrays(
        np.asarray(enc_Wih, np.float32), np.asarray(enc_Whh, np.float32),
        np.asarray(enc_bih, np.float32), np.asarray(enc_bhh, np.float32),
        np.asarray(dec_Wih, np.float32), np.asarray(dec_Whh, np.float32),
        np.asarray(dec_bih, np.float32), np.asarray(dec_bhh, np.float32),
        np.asarray(fc_W, np.float32), np.asarray(fc_b, np.float32))
    in_maps = []
    for i in range(N_CORES):
        xs = x[i * B:(i + 1) * B]
        in_maps.append({**weights, **make_x_arrays(xs, T)})

    res = run_bass_kernel_spmd(nc, in_maps, core_ids=list(range(N_CORES)))
    return postprocess2([r["out"] for r in res.results], T, N_CORES)



# revision 48
# speedup vs baseline: 1.0032x; 1.0032x over previous
"""Self-contained LSTM encoder/decoder kernel for TRN2 (8-core SPMD).

Per-core (data-parallel over batch N=512 -> B=64), the batch is split
into 2 independent chains of BC=32 that are software-pipelined across
the engines: per chain-step
  PE 4 gate matmuls -> ACT tanh(ifgo, one instr) -> DVE uv ->
  DVE C2' -> ACT tanh(c) -> DVE h'.

Math tricks (folded into host-side weight preprocessing):
  - sigma(x) = (tanh(x/2) + 1) / 2  -> ONE tanh over all 4 gates
  - state stored doubled: H2 = 2h, C2 = 2c
      u  = (yi + 1) * yg; v = (yf + 1) * C2
      C2'= 0.5*v + u;  tc = tanh(0.5*C2');  H2' = (yo + 1) * tc
  - decoder feedback folded: gates_{t+1} = h_t @ (fcW.T@dec_Wih.T
    + dec_Whh.T) + bias
  - biases via a ones-row in the h operand (K=101)
  - gate weight chunks padded to 128 columns (keeps LDWEIGHTS 32-col
    aligned; 100-col chunks measured 2x slower)
  - encoder x-projections batched per 8 steps in PSUM group tiles,
    split per-chain so the two chains never share a PSUM bank
  - a sim-time gate (tile_wait_until) paces the Tile scheduler so the
    baked per-engine order interleaves the two chains

Scheduling tricks (each verified on HW):
  - gate-psum tiles are pre-cleared by a dummy matmul with zero
    weights (start=True) so the first real gate MM carries ONLY the
    h2 RAW wait; a second wait would be migrated onto its LDWEIGHTS
    by move_matmul_waits_to_ldweights and stall the weight prefetch
    until h2 is ready (~150ns/step on the recurrence critical path)
  - the decoder FC output COPY (PSUM->SBUF) + DMAs are deferred to a
    late scheduling gate (base+1.25 phases) so the DVE FIFO does not
    head-of-line block the next tail's h2 stt (~190ns/step); the FC
    matmul itself stays early (the next h2 write has a WAR on it)
"""

import numpy as np

import concourse.bass as bass
import concourse.tile as tile
from concourse import bacc, mybir

F32 = mybir.dt.float32
F16 = mybir.dt.float16
AF = mybir.ActivationFunctionType
ALU = mybir.AluOpType

B = 64          # batch per core
BC = 32         # batch per chain
H = 100
F = 150
G = 4 * 100
GW = 128        # weight columns per gate chunk. Padding to 128 keeps the
                # LDWEIGHTS 32-col-group aligned (100-col chunks measured
                # 2x slower: 203ns vs 105ns per load)
GP = 4 * GW
KA = 128
KB = F - KA
T = 300
N_CORES = 8


ENC_PRECLEAR = True
DEC_PRECLEAR = True
H2_ON_GPSIMD = False


def build_nc2(T=300, gates_group=8, gate_ns=940):
    D = T - 1
    nc = bacc.Bacc("TRN2", target_bir_lowering=False, debug=False,
                   enable_asserts=False)

    xa_d = nc.dram_tensor("xa", [KA, T, B], F16, kind="ExternalInput")
    xb_d = nc.dram_tensor("xb", [KB, T, B], F16, kind="ExternalInput")
    w_iha_d = nc.dram_tensor("w_iha", [KA, GP], F16, kind="ExternalInput")
    w_ihb_d = nc.dram_tensor("w_ihb", [KB, GP], F16, kind="ExternalInput")
    w_hh_d = nc.dram_tensor("w_hh", [H + 1, GP], F16, kind="ExternalInput")
    w_d0_d = nc.dram_tensor("w_d0", [H + 1, GP], F16, kind="ExternalInput")
    w_dc_d = nc.dram_tensor("w_dc", [H + 1, GP], F16, kind="ExternalInput")
    w_fc_d = nc.dram_tensor("w_fc", [H + 1, F], F16, kind="ExternalInput")
    out_d = nc.dram_tensor("out", [D, 2, BC, F], F32, kind="ExternalOutput")

    GS = gates_group
    CHUNK_GROUPS = 4
    CHUNK = GS * B * CHUNK_GROUPS

    with tile.TileContext(nc) as tc:
        with (
            tc.tile_pool(name="const", bufs=1) as constp,
            tc.tile_pool(name="state", bufs=1) as statep,
            tc.tile_pool(name="xchunk", bufs=2) as xpool,
            tc.tile_pool(name="ypool", bufs=12) as ypool,
            tc.tile_pool(name="work", bufs=12) as work,
        ):
            w_iha = constp.tile([KA, GP], F16, tag="w_iha")
            w_ihb = constp.tile([KB, GP], F16, tag="w_ihb")
            w_hh = constp.tile([H + 1, GP], F16, tag="w_hh")
            w_d0 = constp.tile([H + 1, GP], F16, tag="w_d0")
            w_dc = constp.tile([H + 1, GP], F16, tag="w_dc")
            w_fc = constp.tile([H + 1, F], F16, tag="w_fc")
            # Zero weights: dummy matmuls pre-clear each gate-psum tile so
            # the real first gate MM carries only the h2 RAW wait (a 2nd
            # wait would migrate onto its LDWEIGHTS and stall the weight
            # prefetch until h2 is ready).
            zw = constp.tile([H + 1, GW], F16, tag="zw")
            # Encoder weights first, spread across engine DGE queues so the
            # startup transfers run in parallel instead of serializing on
            # the Sync queue; decoder weights (w_d0/w_dc/w_fc, not needed
            # for ~670us) are DMA'd after the first x chunk.
            nc.scalar.dma_start(out=w_iha[:], in_=w_iha_d.ap())
            nc.scalar.dma_start(out=w_ihb[:], in_=w_ihb_d.ap())
            nc.gpsimd.dma_start(out=w_hh[:], in_=w_hh_d.ap())

            nc.vector.memset(zw[:], 0.0)

            # h2[part, chain, parity*BC + col]; h_k lives at parity k%2.
            h2 = statep.tile([H + 1, 2, 2 * BC], F16, tag="h2")
            nc.vector.memset(h2[0:96, :, :], 0.0)
            nc.vector.memset(h2[96:H + 1, :, :], 1.0)
            nc.vector.memset(h2[96:H, :, :], 0.0)

            y_cur = [None, None]
            y_nxt = [None, None]
            tc_t = [None, None]
            for X in (0, 1):
                y_cur[X] = ypool.tile([H, 5, BC], F16, tag=f"y{X}",
                                      name=f"y{X}")
                nc.vector.memset(y_cur[X][:, 4, :], 0.0)

            def tg(X, ap_gates):
                nc.scalar.activation(y_cur[X][:, 0:4, :], ap_gates, AF.Tanh)

            def uv_cc(X):
                uv = work.tile([H, 2, BC], F16, tag=f"uv{X}", name=f"uv{X}")
                nc.vector.scalar_tensor_tensor(
                    uv[:], y_cur[X][:, 0:2, :], 1.0, y_cur[X][:, 2:5:2, :],
                    ALU.add, ALU.mult)
                y_nxt[X] = ypool.tile([H, 5, BC], F16, tag=f"y{X}",
                                      name=f"y{X}")
                nc.vector.scalar_tensor_tensor(
                    y_nxt[X][:, 4, :], uv[:, 1, :], 0.5, uv[:, 0, :],
                    ALU.mult, ALU.add)

            h2_eng = nc.gpsimd if H2_ON_GPSIMD else nc.vector

            def tail(X, wp):
                """TC + H2 for chain X's completed step -> h2 parity wp."""
                tc_t[X] = work.tile([H, BC], F16, tag=f"tc{X}",
                                    name=f"tcx{X}")
                nc.scalar.activation(tc_t[X][:], y_nxt[X][:, 4, :], AF.Tanh,
                                     scale=0.5)
                h2_eng.scalar_tensor_tensor(
                    h2[0:H, X, wp * BC:(wp + 1) * BC],
                    y_cur[X][:, 3, :], 1.0, tc_t[X][:],
                    ALU.add, ALU.mult)
                y_cur[X] = y_nxt[X]

            phase_idx = [0]

            def next_base():
                k = phase_idx[0]
                phase_idx[0] += 1
                return k * gate_ns * 1e-6

            def sub(base, frac):
                return tc.tile_wait_until(base + frac * gate_ns * 1e-6)

            # =========== ENCODER ===========
            n_groups = (T + GS - 1) // GS
            CH_STEPS = GS * CHUNK_GROUPS
            with tc.tile_pool(name="epsum", bufs=2, space="PSUM") as epsum:
                chunks = {}

                def load_chunk(ci):
                    s0 = ci * CH_STEPS
                    s1 = min(T, s0 + CH_STEPS)
                    xa_sb = xpool.tile([KA, CH_STEPS, B], F16, tag="xa")
                    xb_sb = xpool.tile([KB, CH_STEPS, B], F16, tag="xb")
                    if ci == 0:
                        # Per-group pieces on alternating DGE queues:
                        # group 0's x-proj matmuls only depend on the
                        # first piece, and pieces transfer in parallel.
                        engs = [nc.sync, nc.gpsimd, nc.scalar]
                        for pi, p0 in enumerate(range(0, s1 - s0, GS)):
                            p1 = min(s1 - s0, p0 + GS)
                            eng = engs[pi % len(engs)]
                            eng.dma_start(
                                out=xa_sb[:, p0:p1, :],
                                in_=xa_d.ap()[:, s0 + p0:s0 + p1, :])
                            eng.dma_start(
                                out=xb_sb[:, p0:p1, :],
                                in_=xb_d.ap()[:, s0 + p0:s0 + p1, :])
                    else:
                        nc.sync.dma_start(out=xa_sb[:, 0:s1 - s0, :],
                                          in_=xa_d.ap()[:, s0:s1, :])
                        nc.sync.dma_start(out=xb_sb[:, 0:s1 - s0, :],
                                          in_=xb_d.ap()[:, s0:s1, :])
                    chunks[ci] = (xa_sb, xb_sb)

                def make_xmm_emitters(g8, pe4):
                    steps = min(GS, T - GS * g8)
                    nc2 = steps * BC
                    ci = g8 // CHUNK_GROUPS
                    offs = (g8 % CHUNK_GROUPS) * GS
                    ems = []
                    if ENC_PRECLEAR:
                        for X in (0, 1):
                            for gi in range(4):
                                def emz(X=X, gi=gi):
                                    nc.tensor.matmul(
                                        pe4[:, X, gi, 0:nc2], zw[:],
                                        w_hh[:, 0:nc2],
                                        start=True, stop=False)
                                ems.append(emz)
                    for X in (0, 1):
                        for gi in range(4):
                            def ema(X=X, gi=gi):
                                xa_sb, _ = chunks[ci]
                                nc.tensor.matmul(
                                    pe4[:, X, gi, 0:nc2],
                                    w_iha[:, gi * GW:(gi + 1) * GW],
                                    xa_sb[:, offs:offs + steps,
                                          X * BC:(X + 1) * BC],
                                    start=not ENC_PRECLEAR, stop=False)
                            def emb(X=X, gi=gi):
                                _, xb_sb = chunks[ci]
                                nc.tensor.matmul(
                                    pe4[:, X, gi, 0:nc2],
                                    w_ihb[:, gi * GW:(gi + 1) * GW],
                                    xb_sb[:, offs:offs + steps,
                                          X * BC:(X + 1) * BC],
                                    start=False, stop=False)
                            ems += [ema, emb]
                    return ems

                load_chunk(0)
                # Decoder weights: issue after the first x chunk so they
                # don't sit ahead of it on the Sync DMA queue.
                nc.sync.dma_start(out=w_d0[:], in_=w_d0_d.ap())
                nc.sync.dma_start(out=w_dc[:], in_=w_dc_d.ap())
                nc.sync.dma_start(out=w_fc[:], in_=w_fc_d.ap())
                pe_cur = epsum.tile([128, 2, 4, GS * BC], F32, tag="pe4")
                for em in make_xmm_emitters(0, pe_cur):
                    em()
                pe_next, next_ems = None, []

                for g8 in range(n_groups):
                    steps = min(GS, T - GS * g8)
                    pci = (g8 + 2) // CHUNK_GROUPS
                    if pci * CHUNK < T * B and pci not in chunks:
                        load_chunk(pci)
                    if g8 + 1 < n_groups:
                        nci = (g8 + 1) // CHUNK_GROUPS
                        if nci not in chunks:
                            load_chunk(nci)
                        pe_next = epsum.tile([128, 2, 4, GS * BC], F32,
                                             tag="pe4")
                        next_ems = make_xmm_emitters(g8 + 1, pe_next)
                    else:
                        pe_next, next_ems = None, []
                    n_ph = 2 * steps
                    per_ph = -(-len(next_ems) // n_ph) if next_ems else 0
                    for s in range(steps):
                        t = GS * g8 + s
                        rp = t % 2
                        for X in (0, 1):
                            Y = 1 - X
                            base = next_base()
                            yt = t - 1 + X
                            with sub(base, 0.0):
                                if yt >= 0:
                                    tail(Y, (yt + 1) % 2)
                                col0 = s * BC
                                for gi in range(4):
                                    nc.tensor.matmul(
                                        pe_cur[:, X, gi, col0:col0 + BC],
                                        w_hh[:, gi * GW:(gi + 1) * GW],
                                        h2[:, X, rp * BC:(rp + 1) * BC],
                                        start=False, stop=True,
                                        skip_group_check=not (s == 0
                                                             and X == 0))
                                ph = 2 * s + X
                                for em in next_ems[ph * per_ph:
                                                   (ph + 1) * per_ph]:
                                    em()
                                tg(X, pe_cur[0:H, X, 0:4, col0:col0 + BC])
                                uv_cc(X)
                    pe_cur = pe_next
                # drain chain 1's step T-1 tail
                tail(1, T % 2)

            # =========== DECODER ===========
            with (
                tc.tile_pool(name="dpsum", bufs=3, space="PSUM") as dpsum,
                tc.tile_pool(name="fpsum", bufs=2, space="PSUM") as fpsum,
            ):
                PO = T % 2

                def emit_fc_mm(X):
                    """FC matmul for chain X (both h2 parities). Must stay
                    early: the next tail's h2 write has a WAR edge on it."""
                    pfc = fpsum.tile([2 * BC, F], F32, tag="pfc")
                    nc.tensor.matmul(pfc[:], h2[:, X, :], w_fc[:],
                                     start=True, stop=True)
                    return pfc

                def emit_fc_out(X, dp, pfc):
                    """PSUM->SBUF copy + output DMAs for steps (dp-1, dp).
                    Deferred to a late scheduling gate so the DVE COPY does
                    not head-of-line block the next tail's h2 stt."""
                    p1 = (dp + PO) % 2
                    ofc = work.tile([2 * BC, F], F32, tag=f"ofc{X}",
                                    name=f"ofc{X}")
                    nc.vector.tensor_copy(ofc[:], pfc[:])
                    nc.sync.dma_start(out=out_d.ap()[dp - 1, X],
                                      in_=ofc[p1 * BC:(p1 + 1) * BC, :])
                    nc.sync.dma_start(out=out_d.ap()[dp - 2, X],
                                      in_=ofc[(1 - p1) * BC:(2 - p1) * BC, :])

                for d in range(1, D + 1):
                    wd = w_d0 if d == 1 else w_dc
                    rp = (d - 1 + PO) % 2
                    for X in (0, 1):
                        Y = 1 - X
                        base = next_base()
                        yt = d - 1 + X
                        pfc = None
                        with sub(base, 0.0):
                            if (X == 1) or (d >= 2):
                                tail(Y, (yt + PO) % 2)
                            pd = dpsum.tile([128, 4 * BC], F32,
                                            tag=f"pd{X}", name=f"pd{X}")
                            if DEC_PRECLEAR:
                                nc.tensor.matmul(
                                    pd[:, :], zw[:], w_hh[:, 0:4 * BC],
                                    start=True, stop=False)
                            for gi in range(4):
                                nc.tensor.matmul(
                                    pd[:, gi * BC:(gi + 1) * BC],
                                    wd[:, gi * GW:(gi + 1) * GW],
                                    h2[:, X, rp * BC:(rp + 1) * BC],
                                    start=(gi == 0 and not DEC_PRECLEAR),
                                    stop=gi == 3)
                            if d >= 3 and d % 2 == 1:
                                pfc = emit_fc_mm(X)
                            tg(X, pd[0:H, :])
                            uv_cc(X)
                        if pfc is not None:
                            with sub(base, 1.25):
                                emit_fc_out(X, d - 1, pfc)
                # drain chain 1's step D tail
                tail(1, (D + PO) % 2)
                for X in (0, 1):
                    if D % 2 == 0:
                        pfc = emit_fc_mm(X)
                        emit_fc_out(X, D, pfc)
                    else:
                        pD = (D + PO) % 2
                        pfc = fpsum.tile([2 * BC, F], F32, tag="pfc")
                        nc.tensor.matmul(pfc[0:BC, :],
                                         h2[:, X, pD * BC:(pD + 1) * BC],
                                         w_fc[:], start=True, stop=True)
                        ofc = work.tile([2 * BC, F], F32, tag=f"ofc{X}",
                                        name=f"ofc{X}")
                        nc.vector.tensor_copy(ofc[0:BC, :], pfc[0:BC, :])
                        nc.sync.dma_start(out=out_d.ap()[D - 1, X],
                                          in_=ofc[0:BC, :])

    nc.compile()
    return nc


# ======================= host pre/post =======================

def _colscale():
    s = np.ones(G, np.float32)
    s[0:100] = 0.5    # i
    s[100:200] = 0.5  # f
    s[300:400] = 0.5  # o
    return s


def _pad_gates(w):
    K = w.shape[0]
    out = np.zeros((K, GP), w.dtype)
    for gi in range(4):
        out[:, gi * GW:gi * GW + 100] = w[:, gi * 100:(gi + 1) * 100]
    return out


def make_weight_arrays(enc_Wih, enc_Whh, enc_bih, enc_bhh,
                       dec_Wih, dec_Whh, dec_bih, dec_bhh, fc_W, fc_b):
    cs = _colscale()
    f64 = np.float64
    w_ih = (enc_Wih.T.astype(f64) * cs).astype(np.float32)
    w_hh = np.vstack([enc_Whh.T.astype(f64) * 0.5,
                      (enc_bih + enc_bhh).astype(f64)[None, :]]) * cs
    w_d0 = np.vstack([dec_Whh.T.astype(f64) * 0.5,
                      (dec_bih + dec_bhh).astype(f64)[None, :]]) * cs
    combo = fc_W.T.astype(f64) @ dec_Wih.T.astype(f64) + dec_Whh.T.astype(f64)
    bias_c = (fc_b.astype(f64) @ dec_Wih.T.astype(f64)
              + dec_bih.astype(f64) + dec_bhh.astype(f64))
    w_dc = np.vstack([combo * 0.5, bias_c[None, :]]) * cs
    w_fc = np.vstack([fc_W.T.astype(f64) * 0.5, fc_b.astype(f64)[None, :]])
    return {
        "w_iha": _pad_gates(w_ih[0:KA]).astype(np.float16),
        "w_ihb": _pad_gates(w_ih[KA:F]).astype(np.float16),
        "w_hh": _pad_gates(w_hh).astype(np.float16),
        "w_d0": _pad_gates(w_d0).astype(np.float16),
        "w_dc": _pad_gates(w_dc).astype(np.float16),
        "w_fc": w_fc.astype(np.float16),
    }


def make_x_arrays(x_shard, T):
    """x_shard (B, 3, T, 25, 2) -> xa (128, T, B), xb (22, T, B) fp16."""
    xt = np.ascontiguousarray(
        x_shard.transpose(1, 3, 4, 2, 0)).reshape(F, T, B)
    xt = xt.astype(np.float16)
    return {"xa": np.ascontiguousarray(xt[0:KA]),
            "xb": np.ascontiguousarray(xt[KA:F])}


def postprocess2(core_outs, T, n_cores=8):
    """core_outs: list of (D, 2, BC, F) -> full (N, 3, T, 25, 2)."""
    N = n_cores * B
    full = np.zeros((N, 3, T, 25, 2), np.float32)
    for i, o in enumerate(core_outs):
        ob = o.reshape(T - 1, B, 3, 25, 2).transpose(1, 2, 0, 3, 4)
        full[i * B:(i + 1) * B, :, 1:T] = ob
    return full


# ======================= self-contained kernel entry =======================

_NC_CACHE = {}


def _get_nc():
    if "nc" not in _NC_CACHE:
        _NC_CACHE["nc"] = build_nc2(T=T)
    return _NC_CACHE["nc"]


def kernel(x, enc_Wih, enc_Whh, enc_bih, enc_bhh,
           dec_Wih, dec_Whh, dec_bih, dec_bhh, fc_W, fc_b):
    from concourse.bass_utils import run_bass_kernel_spmd

    x = np.asarray(x, np.float32)
    nc = _get_nc()
    weights = make_weight_arrays(
        np.asarray(enc_Wih, np.float32), np.asarray(enc_Whh, np.float32),
        np.asarray(enc_bih, np.float32), np.asarray(enc_bhh, np.float32),
        np.asarray(dec_Wih, np.float32), np.asarray(dec_Whh, np.float32),
        np.asarray(dec_bih, np.float32), np.asarray(dec_bhh, np.float32),
        np.asarray(fc_W, np.float32), np.asarray(fc_b, np.float32))
    in_maps = []
    for i in range(N_CORES):
        xs = x[i * B:(i + 1) * B]
        in_maps.append({**weights, **make_x_arrays(xs, T)})

    res = run_bass_kernel_spmd(nc, in_maps, core_ids=list(range(N_CORES)))
    return postprocess2([r["out"] for r in res.results], T, N_CORES)



# revision 50
# speedup vs baseline: 1.0040x; 1.0009x over previous
"""Self-contained LSTM encoder/decoder kernel for TRN2 (8-core SPMD).

Per-core (data-parallel over batch N=512 -> B=64), the batch is split
into 2 independent chains of BC=32 that are software-pipelined across
the engines: per chain-step
  PE 4 gate matmuls -> ACT tanh(ifgo, one instr) -> DVE uv ->
  DVE C2' -> ACT tanh(c) -> DVE h'.

Math tricks (folded into host-side weight preprocessing):
  - sigma(x) = (tanh(x/2) + 1) / 2  -> ONE tanh over all 4 gates
  - state stored doubled: H2 = 2h, C2 = 2c
      u  = (yi + 1) * yg; v = (yf + 1) * C2
      C2'= 0.5*v + u;  tc = tanh(0.5*C2');  H2' = (yo + 1) * tc
  - decoder feedback folded: gates_{t+1} = h_t @ (fcW.T@dec_Wih.T
    + dec_Whh.T) + bias
  - biases via a ones-row in the h operand (K=101)
  - gate weight chunks padded to 128 columns (keeps LDWEIGHTS 32-col
    aligned; 100-col chunks measured 2x slower)
  - encoder x-projections batched per 8 steps in PSUM group tiles,
    split per-chain so the two chains never share a PSUM bank
  - a sim-time gate (tile_wait_until) paces the Tile scheduler so the
    baked per-engine order interleaves the two chains

Scheduling tricks (each verified on HW):
  - gate-psum tiles are pre-cleared by a dummy matmul with zero
    weights (start=True) so the first real gate MM carries ONLY the
    h2 RAW wait; a second wait would be migrated onto its LDWEIGHTS
    by move_matmul_waits_to_ldweights and stall the weight prefetch
    until h2 is ready (~150ns/step on the recurrence critical path)
  - the decoder FC output COPY (PSUM->SBUF) + DMAs are deferred to a
    late scheduling gate (base+1.25 phases) so the DVE FIFO does not
    head-of-line block the next tail's h2 stt (~190ns/step); the FC
    matmul itself stays early (the next h2 write has a WAR on it)
"""

import numpy as np

import concourse.bass as bass
import concourse.tile as tile
from concourse import bacc, mybir

F32 = mybir.dt.float32
F16 = mybir.dt.float16
AF = mybir.ActivationFunctionType
ALU = mybir.AluOpType

B = 64          # batch per core
BC = 32         # batch per chain
H = 100
F = 150
G = 4 * 100
GW = 128        # weight columns per gate chunk. Padding to 128 keeps the
                # LDWEIGHTS 32-col-group aligned (100-col chunks measured
                # 2x slower: 203ns vs 105ns per load)
GP = 4 * GW
KA = 128
KB = F - KA
T = 300
N_CORES = 8


ENC_PRECLEAR = True
DEC_PRECLEAR = True
H2_ON_GPSIMD = False


def build_nc2(T=300, gates_group=8, gate_ns=940):
    D = T - 1
    nc = bacc.Bacc("TRN2", target_bir_lowering=False, debug=False,
                   enable_asserts=False)

    xa_d = nc.dram_tensor("xa", [KA, T, B], F16, kind="ExternalInput")
    xb_d = nc.dram_tensor("xb", [KB, T, B], F16, kind="ExternalInput")
    w_iha_d = nc.dram_tensor("w_iha", [KA, GP], F16, kind="ExternalInput")
    w_ihb_d = nc.dram_tensor("w_ihb", [KB, GP], F16, kind="ExternalInput")
    w_hh_d = nc.dram_tensor("w_hh", [H + 1, GP], F16, kind="ExternalInput")
    w_d0_d = nc.dram_tensor("w_d0", [H + 1, GP], F16, kind="ExternalInput")
    w_dc_d = nc.dram_tensor("w_dc", [H + 1, GP], F16, kind="ExternalInput")
    w_fc_d = nc.dram_tensor("w_fc", [H + 1, F], F16, kind="ExternalInput")
    out_d = nc.dram_tensor("out", [D, 2, BC, F], F32, kind="ExternalOutput")

    GS = gates_group
    CHUNK_GROUPS = 4
    CHUNK = GS * B * CHUNK_GROUPS

    with tile.TileContext(nc) as tc:
        with (
            tc.tile_pool(name="const", bufs=1) as constp,
            tc.tile_pool(name="state", bufs=1) as statep,
            tc.tile_pool(name="xchunk", bufs=2) as xpool,
            tc.tile_pool(name="ypool", bufs=12) as ypool,
            tc.tile_pool(name="work", bufs=12) as work,
        ):
            w_iha = constp.tile([KA, GP], F16, tag="w_iha")
            w_ihb = constp.tile([KB, GP], F16, tag="w_ihb")
            w_hh = constp.tile([H + 1, GP], F16, tag="w_hh")
            w_d0 = constp.tile([H + 1, GP], F16, tag="w_d0")
            w_dc = constp.tile([H + 1, GP], F16, tag="w_dc")
            w_fc = constp.tile([H + 1, F], F16, tag="w_fc")
            # Zero weights: dummy matmuls pre-clear each gate-psum tile so
            # the real first gate MM carries only the h2 RAW wait (a 2nd
            # wait would migrate onto its LDWEIGHTS and stall the weight
            # prefetch until h2 is ready).
            zw = constp.tile([H + 1, GW], F16, tag="zw")
            # Encoder weights first, spread across engine DGE queues so the
            # startup transfers run in parallel instead of serializing on
            # the Sync queue; decoder weights (w_d0/w_dc/w_fc, not needed
            # for ~670us) are DMA'd after the first x chunk.
            nc.scalar.dma_start(out=w_iha[:], in_=w_iha_d.ap())
            nc.scalar.dma_start(out=w_ihb[:], in_=w_ihb_d.ap())
            nc.gpsimd.dma_start(out=w_hh[:], in_=w_hh_d.ap())

            nc.vector.memset(zw[:], 0.0)

            # h2[part, chain, parity*BC + col]; h_k lives at parity k%2.
            h2 = statep.tile([H + 1, 2, 2 * BC], F16, tag="h2")
            nc.vector.memset(h2[0:96, :, :], 0.0)
            nc.vector.memset(h2[96:H + 1, :, :], 1.0)
            nc.vector.memset(h2[96:H, :, :], 0.0)

            y_cur = [None, None]
            y_nxt = [None, None]
            tc_t = [None, None]
            for X in (0, 1):
                y_cur[X] = ypool.tile([H, 5, BC], F16, tag=f"y{X}",
                                      name=f"y{X}")
                nc.vector.memset(y_cur[X][:, 4, :], 0.0)

            def tg(X, ap_gates):
                nc.scalar.activation(y_cur[X][:, 0:4, :], ap_gates, AF.Tanh)

            def uv_cc(X):
                uv = work.tile([H, 2, BC], F16, tag=f"uv{X}", name=f"uv{X}")
                nc.vector.scalar_tensor_tensor(
                    uv[:], y_cur[X][:, 0:2, :], 1.0, y_cur[X][:, 2:5:2, :],
                    ALU.add, ALU.mult)
                y_nxt[X] = ypool.tile([H, 5, BC], F16, tag=f"y{X}",
                                      name=f"y{X}")
                nc.vector.scalar_tensor_tensor(
                    y_nxt[X][:, 4, :], uv[:, 1, :], 0.5, uv[:, 0, :],
                    ALU.mult, ALU.add)

            h2_eng = nc.gpsimd if H2_ON_GPSIMD else nc.vector

            def tail(X, wp):
                """TC + H2 for chain X's completed step -> h2 parity wp."""
                tc_t[X] = work.tile([H, BC], F16, tag=f"tc{X}",
                                    name=f"tcx{X}")
                nc.scalar.activation(tc_t[X][:], y_nxt[X][:, 4, :], AF.Tanh,
                                     scale=0.5)
                h2_eng.scalar_tensor_tensor(
                    h2[0:H, X, wp * BC:(wp + 1) * BC],
                    y_cur[X][:, 3, :], 1.0, tc_t[X][:],
                    ALU.add, ALU.mult)
                y_cur[X] = y_nxt[X]

            phase_idx = [0]

            def next_base():
                k = phase_idx[0]
                phase_idx[0] += 1
                return k * gate_ns * 1e-6

            def sub(base, frac):
                return tc.tile_wait_until(base + frac * gate_ns * 1e-6)

            # =========== ENCODER ===========
            n_groups = (T + GS - 1) // GS
            CH_STEPS = GS * CHUNK_GROUPS
            with tc.tile_pool(name="epsum", bufs=2, space="PSUM") as epsum:
                chunks = {}

                def load_chunk(ci):
                    s0 = ci * CH_STEPS
                    s1 = min(T, s0 + CH_STEPS)
                    xa_sb = xpool.tile([KA, CH_STEPS, B], F16, tag="xa")
                    xb_sb = xpool.tile([KB, CH_STEPS, B], F16, tag="xb")
                    if ci == 0:
                        # Per-group pieces on alternating DGE queues:
                        # group 0's x-proj matmuls only depend on the
                        # first piece, and pieces transfer in parallel.
                        engs = [nc.sync, nc.gpsimd, nc.scalar]
                        for pi, p0 in enumerate(range(0, s1 - s0, GS)):
                            p1 = min(s1 - s0, p0 + GS)
                            eng = engs[pi % len(engs)]
                            eng.dma_start(
                                out=xa_sb[:, p0:p1, :],
                                in_=xa_d.ap()[:, s0 + p0:s0 + p1, :])
                            eng.dma_start(
                                out=xb_sb[:, p0:p1, :],
                                in_=xb_d.ap()[:, s0 + p0:s0 + p1, :])
                    else:
                        nc.sync.dma_start(out=xa_sb[:, 0:s1 - s0, :],
                                          in_=xa_d.ap()[:, s0:s1, :])
                        nc.sync.dma_start(out=xb_sb[:, 0:s1 - s0, :],
                                          in_=xb_d.ap()[:, s0:s1, :])
                    chunks[ci] = (xa_sb, xb_sb)

                def make_xmm_emitters(g8, pe4):
                    steps = min(GS, T - GS * g8)
                    nc2 = steps * BC
                    ci = g8 // CHUNK_GROUPS
                    offs = (g8 % CHUNK_GROUPS) * GS
                    ems = []
                    if ENC_PRECLEAR:
                        for X in (0, 1):
                            for gi in range(4):
                                def emz(X=X, gi=gi):
                                    nc.tensor.matmul(
                                        pe4[:, X, gi, 0:nc2], zw[:],
                                        w_hh[:, 0:nc2],
                                        start=True, stop=False)
                                ems.append(emz)
                    for X in (0, 1):
                        for gi in range(4):
                            def ema(X=X, gi=gi):
                                xa_sb, _ = chunks[ci]
                                nc.tensor.matmul(
                                    pe4[:, X, gi, 0:nc2],
                                    w_iha[:, gi * GW:(gi + 1) * GW],
                                    xa_sb[:, offs:offs + steps,
                                          X * BC:(X + 1) * BC],
                                    start=not ENC_PRECLEAR, stop=False)
                            def emb(X=X, gi=gi):
                                _, xb_sb = chunks[ci]
                                nc.tensor.matmul(
                                    pe4[:, X, gi, 0:nc2],
                                    w_ihb[:, gi * GW:(gi + 1) * GW],
                                    xb_sb[:, offs:offs + steps,
                                          X * BC:(X + 1) * BC],
                                    start=False, stop=False)
                            ems += [ema, emb]
                    return ems

                load_chunk(0)
                # Decoder weights: issue after the first x chunk so they
                # don't sit ahead of it on the Sync DMA queue.
                nc.sync.dma_start(out=w_d0[:], in_=w_d0_d.ap())
                nc.sync.dma_start(out=w_dc[:], in_=w_dc_d.ap())
                nc.sync.dma_start(out=w_fc[:], in_=w_fc_d.ap())
                pe_cur = epsum.tile([128, 2, 4, GS * BC], F32, tag="pe4")
                for em in make_xmm_emitters(0, pe_cur):
                    em()
                pe_next, next_ems = None, []

                for g8 in range(n_groups):
                    steps = min(GS, T - GS * g8)
                    pci = (g8 + 2) // CHUNK_GROUPS
                    if pci * CHUNK < T * B and pci not in chunks:
                        load_chunk(pci)
                    if g8 + 1 < n_groups:
                        nci = (g8 + 1) // CHUNK_GROUPS
                        if nci not in chunks:
                            load_chunk(nci)
                        pe_next = epsum.tile([128, 2, 4, GS * BC], F32,
                                             tag="pe4")
                        next_ems = make_xmm_emitters(g8 + 1, pe_next)
                    else:
                        pe_next, next_ems = None, []
                    n_ph = 2 * steps
                    per_ph = -(-len(next_ems) // n_ph) if next_ems else 0
                    for s in range(steps):
                        t = GS * g8 + s
                        rp = t % 2
                        for X in (0, 1):
                            Y = 1 - X
                            base = next_base()
                            yt = t - 1 + X
                            with sub(base, 0.0):
                                if yt >= 0:
                                    tail(Y, (yt + 1) % 2)
                                col0 = s * BC
                                for gi in range(4):
                                    nc.tensor.matmul(
                                        pe_cur[:, X, gi, col0:col0 + BC],
                                        w_hh[:, gi * GW:(gi + 1) * GW],
                                        h2[:, X, rp * BC:(rp + 1) * BC],
                                        start=False, stop=True,
                                        skip_group_check=not (s == 0
                                                             and X == 0))
                                ph = 2 * s + X
                                for em in next_ems[ph * per_ph:
                                                   (ph + 1) * per_ph]:
                                    em()
                                tg(X, pe_cur[0:H, X, 0:4, col0:col0 + BC])
                                uv_cc(X)
                    pe_cur = pe_next
                # drain chain 1's step T-1 tail
                tail(1, T % 2)

            # =========== DECODER ===========
            with (
                tc.tile_pool(name="dpsum", bufs=3, space="PSUM") as dpsum,
                tc.tile_pool(name="fpsum", bufs=2, space="PSUM") as fpsum,
            ):
                PO = T % 2

                def emit_fc_mm(X):
                    """FC matmul for chain X (both h2 parities). Must stay
                    early: the next tail's h2 write has a WAR edge on it."""
                    pfc = fpsum.tile([2 * BC, F], F32, tag="pfc")
                    nc.tensor.matmul(pfc[:], h2[:, X, :], w_fc[:],
                                     start=True, stop=True)
                    return pfc

                def emit_fc_out(X, dp, pfc):
                    """PSUM->SBUF copy + output DMAs for steps (dp-1, dp).
                    Deferred to a late scheduling gate so the DVE COPY does
                    not head-of-line block the next tail's h2 stt."""
                    p1 = (dp + PO) % 2
                    ofc = work.tile([2 * BC, F], F32, tag=f"ofc{X}",
                                    name=f"ofc{X}")
                    nc.vector.tensor_copy(ofc[:], pfc[:])
                    nc.sync.dma_start(out=out_d.ap()[dp - 1, X],
                                      in_=ofc[p1 * BC:(p1 + 1) * BC, :])
                    nc.sync.dma_start(out=out_d.ap()[dp - 2, X],
                                      in_=ofc[(1 - p1) * BC:(2 - p1) * BC, :])

                for d in range(1, D + 1):
                    wd = w_d0 if d == 1 else w_dc
                    rp = (d - 1 + PO) % 2
                    for X in (0, 1):
                        Y = 1 - X
                        base = next_base()
                        yt = d - 1 + X
                        pfc = None
                        with sub(base, 0.0):
                            if (X == 1) or (d >= 2):
                                tail(Y, (yt + PO) % 2)
                            pd = dpsum.tile([128, 4 * BC], F32,
                                            tag=f"pd{X}", name=f"pd{X}")
                            if DEC_PRECLEAR:
                                nc.tensor.matmul(
                                    pd[:, :], zw[:], w_hh[:, 0:4 * BC],
                                    start=True, stop=False)
                            for gi in range(4):
                                nc.tensor.matmul(
                                    pd[:, gi * BC:(gi + 1) * BC],
                                    wd[:, gi * GW:(gi + 1) * GW],
                                    h2[:, X, rp * BC:(rp + 1) * BC],
                                    start=(gi == 0 and not DEC_PRECLEAR),
                                    stop=gi == 3)
                            if d >= 3 and d % 2 == 1:
                                pfc = emit_fc_mm(X)
                            tg(X, pd[0:H, :])
                            uv_cc(X)
                        if pfc is not None:
                            with sub(base, 1.25):
                                emit_fc_out(X, d - 1, pfc)
                # drain chain 1's step D tail
                tail(1, (D + PO) % 2)
                for X in (0, 1):
                    if D % 2 == 0:
                        pfc = emit_fc_mm(X)
                        emit_fc_out(X, D, pfc)
                    else:
                        pD = (D + PO) % 2
                        pfc = fpsum.tile([2 * BC, F], F32, tag="pfc")
                        nc.tensor.matmul(pfc[0:BC, :],
                                         h2[:, X, pD * BC:(pD + 1) * BC],
                                         w_fc[:], start=True, stop=True)
                        ofc = work.tile([2 * BC, F], F32, tag=f"ofc{X}",
                                        name=f"ofc{X}")
                        nc.vector.tensor_copy(ofc[0:BC, :], pfc[0:BC, :])
                        nc.sync.dma_start(out=out_d.ap()[D - 1, X],
                                          in_=ofc[0:BC, :])

    nc.compile()
    return nc


# ======================= host pre/post =======================

def _colscale():
    s = np.ones(G, np.float32)
    s[0:100] = 0.5    # i
    s[100:200] = 0.5  # f
    s[300:400] = 0.5  # o
    return s


def _pad_gates(w):
    K = w.shape[0]
    out = np.zeros((K, GP), w.dtype)
    for gi in range(4):
        out[:, gi * GW:gi * GW + 100] = w[:, gi * 100:(gi + 1) * 100]
    return out


def make_weight_arrays(enc_Wih, enc_Whh, enc_bih, enc_bhh,
                       dec_Wih, dec_Whh, dec_bih, dec_bhh, fc_W, fc_b):
    cs = _colscale()
    f64 = np.float64
    w_ih = (enc_Wih.T.astype(f64) * cs).astype(np.float32)
    w_hh = np.vstack([enc_Whh.T.astype(f64) * 0.5,
                      (enc_bih + enc_bhh).astype(f64)[None, :]]) * cs
    w_d0 = np.vstack([dec_Whh.T.astype(f64) * 0.5,
                      (dec_bih + dec_bhh).astype(f64)[None, :]]) * cs
    combo = fc_W.T.astype(f64) @ dec_Wih.T.astype(f64) + dec_Whh.T.astype(f64)
    bias_c = (fc_b.astype(f64) @ dec_Wih.T.astype(f64)
              + dec_bih.astype(f64) + dec_bhh.astype(f64))
    w_dc = np.vstack([combo * 0.5, bias_c[None, :]]) * cs
    w_fc = np.vstack([fc_W.T.astype(f64) * 0.5, fc_b.astype(f64)[None, :]])
    return {
        "w_iha": _pad_gates(w_ih[0:KA]).astype(np.float16),
        "w_ihb": _pad_gates(w_ih[KA:F]).astype(np.float16),
        "w_hh": _pad_gates(w_hh).astype(np.float16),
        "w_d0": _pad_gates(w_d0).astype(np.float16),
        "w_dc": _pad_gates(w_dc).astype(np.float16),
        "w_fc": w_fc.astype(np.float16),
    }


def make_x_arrays(x_shard, T):
    """x_shard (B, 3, T, 25, 2) -> xa (128, T, B), xb (22, T, B) fp16."""
    xt = np.ascontiguousarray(
        x_shard.transpose(1, 3, 4, 2, 0)).reshape(F, T, B)
    xt = xt.astype(np.float16)
    return {"xa": np.ascontiguousarray(xt[0:KA]),
            "xb": np.ascontiguousarray(xt[KA:F])}


def postprocess2(core_outs, T, n_cores=8):
    """core_outs: list of (D, 2, BC, F) -> full (N, 3, T, 25, 2)."""
    N = n_cores * B
    full = np.zeros((N, 3, T, 25, 2), np.float32)
    for i, o in enumerate(core_outs):
        ob = o.reshape(T - 1, B, 3, 25, 2).transpose(1, 2, 0, 3, 4)
        full[i * B:(i + 1) * B, :, 1:T] = ob
    return full


# ======================= self-contained kernel entry =======================

_NC_CACHE = {}


def _get_nc():
    if "nc" not in _NC_CACHE:
        _NC_CACHE["nc"] = build_nc2(T=T)
    return _NC_CACHE["nc"]


def kernel(x, enc_Wih, enc_Whh, enc_bih, enc_bhh,
           dec_Wih, dec_Whh, dec_bih, dec_bhh, fc_W, fc_b):
    from concourse.bass_utils import run_bass_kernel_spmd

    x = np.asarray(x, np.float32)
    nc = _get_nc()
    weights = make_weight_arrays(
        np.asarray(enc_Wih, np.float32), np.asarray(enc_Whh, np.float32),
        np.asarray(enc_bih, np.float32), np.asarray(enc_bhh, np.float32),
        np.asarray(dec_Wih, np.float32), np.asarray(dec_Whh, np.float32),
        np.asarray(dec_bih, np.float32), np.asarray(dec_bhh, np.float32),
        np.asarray(fc_W, np.float32), np.asarray(fc_b, np.float32))
    in_maps = []
    for i in range(N_CORES):
        xs = x[i * B:(i + 1) * B]
        in_maps.append({**weights, **make_x_arrays(xs, T)})

    res = run_bass_kernel_spmd(nc, in_maps, core_ids=list(range(N_CORES)))
    return postprocess2([r["out"] for r in res.results], T, N_CORES)



# revision 51
# speedup vs baseline: 1.0043x; 1.0003x over previous
"""Self-contained LSTM encoder/decoder kernel for TRN2 (8-core SPMD).

Per-core (data-parallel over batch N=512 -> B=64), the batch is split
into 2 independent chains of BC=32 that are software-pipelined across
the engines: per chain-step
  PE 4 gate matmuls -> ACT tanh(ifgo, one instr) -> DVE uv ->
  DVE C2' -> ACT tanh(c) -> DVE h'.

Math tricks (folded into host-side weight preprocessing):
  - sigma(x) = (tanh(x/2) + 1) / 2  -> ONE tanh over all 4 gates
  - state stored doubled: H2 = 2h, C2 = 2c
      u  = (yi + 1) * yg; v = (yf + 1) * C2
      C2'= 0.5*v + u;  tc = tanh(0.5*C2');  H2' = (yo + 1) * tc
  - decoder feedback folded: gates_{t+1} = h_t @ (fcW.T@dec_Wih.T
    + dec_Whh.T) + bias
  - biases via a ones-row in the h operand (K=101)
  - gate weight chunks padded to 128 columns (keeps LDWEIGHTS 32-col
    aligned; 100-col chunks measured 2x slower)
  - encoder x-projections batched per 8 steps in PSUM group tiles,
    split per-chain so the two chains never share a PSUM bank
  - a sim-time gate (tile_wait_until) paces the Tile scheduler so the
    baked per-engine order interleaves the two chains

Scheduling tricks (each verified on HW):
  - gate-psum tiles are pre-cleared by a dummy matmul with zero
    weights (start=True) so the first real gate MM carries ONLY the
    h2 RAW wait; a second wait would be migrated onto its LDWEIGHTS
    by move_matmul_waits_to_ldweights and stall the weight prefetch
    until h2 is ready (~150ns/step on the recurrence critical path)
  - the decoder FC output COPY (PSUM->SBUF) + DMAs are deferred to a
    late scheduling gate (base+1.25 phases) so the DVE FIFO does not
    head-of-line block the next tail's h2 stt (~190ns/step); the FC
    matmul itself stays early (the next h2 write has a WAR on it)
"""

import numpy as np

import concourse.bass as bass
import concourse.tile as tile
from concourse import bacc, mybir

F32 = mybir.dt.float32
F16 = mybir.dt.float16
AF = mybir.ActivationFunctionType
ALU = mybir.AluOpType

B = 64          # batch per core
BC = 32         # batch per chain
H = 100
F = 150
G = 4 * 100
GW = 128        # weight columns per gate chunk. Padding to 128 keeps the
                # LDWEIGHTS 32-col-group aligned (100-col chunks measured
                # 2x slower: 203ns vs 105ns per load)
GP = 4 * GW
KA = 128
KB = F - KA
T = 300
N_CORES = 8


ENC_PRECLEAR = True
DEC_PRECLEAR = True
H2_ON_GPSIMD = False


def build_nc2(T=300, gates_group=8, gate_ns=940):
    D = T - 1
    nc = bacc.Bacc("TRN2", target_bir_lowering=False, debug=False,
                   enable_asserts=False)

    xa_d = nc.dram_tensor("xa", [KA, T, B], F16, kind="ExternalInput")
    xb_d = nc.dram_tensor("xb", [KB, T, B], F16, kind="ExternalInput")
    w_iha_d = nc.dram_tensor("w_iha", [KA, GP], F16, kind="ExternalInput")
    w_ihb_d = nc.dram_tensor("w_ihb", [KB, GP], F16, kind="ExternalInput")
    w_hh_d = nc.dram_tensor("w_hh", [H + 1, GP], F16, kind="ExternalInput")
    w_d0_d = nc.dram_tensor("w_d0", [H + 1, GP], F16, kind="ExternalInput")
    w_dc_d = nc.dram_tensor("w_dc", [H + 1, GP], F16, kind="ExternalInput")
    w_fc_d = nc.dram_tensor("w_fc", [H + 1, F], F16, kind="ExternalInput")
    out_d = nc.dram_tensor("out", [D, 2, BC, F], F32, kind="ExternalOutput")

    GS = gates_group
    CHUNK_GROUPS = 4
    CHUNK = GS * B * CHUNK_GROUPS

    with tile.TileContext(nc) as tc:
        with (
            tc.tile_pool(name="const", bufs=1) as constp,
            tc.tile_pool(name="state", bufs=1) as statep,
            tc.tile_pool(name="xchunk", bufs=2) as xpool,
            tc.tile_pool(name="ypool", bufs=12) as ypool,
            tc.tile_pool(name="work", bufs=12) as work,
        ):
            w_iha = constp.tile([KA, GP], F16, tag="w_iha")
            w_ihb = constp.tile([KB, GP], F16, tag="w_ihb")
            w_hh = constp.tile([H + 1, GP], F16, tag="w_hh")
            w_d0 = constp.tile([H + 1, GP], F16, tag="w_d0")
            w_dc = constp.tile([H + 1, GP], F16, tag="w_dc")
            w_fc = constp.tile([H + 1, F], F16, tag="w_fc")
            # Zero weights: dummy matmuls pre-clear each gate-psum tile so
            # the real first gate MM carries only the h2 RAW wait (a 2nd
            # wait would migrate onto its LDWEIGHTS and stall the weight
            # prefetch until h2 is ready).
            zw = constp.tile([H + 1, GW], F16, tag="zw")
            # Encoder weights first, spread across engine DGE queues so the
            # startup transfers run in parallel instead of serializing on
            # the Sync queue; decoder weights (w_d0/w_dc/w_fc, not needed
            # for ~670us) are DMA'd after the first x chunk.
            nc.scalar.dma_start(out=w_iha[:], in_=w_iha_d.ap())
            nc.scalar.dma_start(out=w_ihb[:], in_=w_ihb_d.ap())
            nc.gpsimd.dma_start(out=w_hh[:], in_=w_hh_d.ap())

            nc.vector.memset(zw[:], 0.0)

            # h2[part, chain, parity*BC + col]; h_k lives at parity k%2.
            h2 = statep.tile([H + 1, 2, 2 * BC], F16, tag="h2")
            nc.vector.memset(h2[0:96, :, :], 0.0)
            nc.vector.memset(h2[96:H + 1, :, :], 1.0)
            nc.vector.memset(h2[96:H, :, :], 0.0)

            # Flat y layout, slot order [o, i, f, g, C2] (weight chunks
            # reordered host-side to o,i,f,g): every stt operand below is a
            # flat contiguous unit-stride slice, giving the DVE 2x_1p
            # auto-detect its best shot (the old strided [2,32] in1 AP
            # measured at 1x mode).
            y_cur = [None, None]
            y_nxt = [None, None]
            tc_t = [None, None]
            for X in (0, 1):
                y_cur[X] = ypool.tile([H, 5 * BC], F16, tag=f"y{X}",
                                      name=f"y{X}")
                nc.vector.memset(y_cur[X][:, 4 * BC:5 * BC], 0.0)

            def tg(X, ap_gates):
                nc.scalar.activation(y_cur[X][:, 0:4 * BC], ap_gates,
                                     AF.Tanh)

            def uv_cc(X):
                uv = work.tile([H, 2 * BC], F16, tag=f"uv{X}", name=f"uv{X}")
                # in0 = [i, f] (slots 1-2), in1 = [g, C2] (slots 3-4):
                # u = (i+1)*g, v = (f+1)*C2
                nc.vector.scalar_tensor_tensor(
                    uv[:], y_cur[X][:, BC:3 * BC], 1.0,
                    y_cur[X][:, 3 * BC:5 * BC],
                    ALU.add, ALU.mult)
                y_nxt[X] = ypool.tile([H, 5 * BC], F16, tag=f"y{X}",
                                      name=f"y{X}")
                nc.vector.scalar_tensor_tensor(
                    y_nxt[X][:, 4 * BC:5 * BC], uv[:, BC:2 * BC], 0.5,
                    uv[:, 0:BC],
                    ALU.mult, ALU.add)

            h2_eng = nc.gpsimd if H2_ON_GPSIMD else nc.vector

            def tail(X, wp):
                """TC + H2 for chain X's completed step -> h2 parity wp."""
                tc_t[X] = work.tile([H, BC], F16, tag=f"tc{X}",
                                    name=f"tcx{X}")
                nc.scalar.activation(tc_t[X][:], y_nxt[X][:, 4 * BC:5 * BC],
                                     AF.Tanh, scale=0.5)
                h2_eng.scalar_tensor_tensor(
                    h2[0:H, X, wp * BC:(wp + 1) * BC],
                    y_cur[X][:, 0:BC], 1.0, tc_t[X][:],
                    ALU.add, ALU.mult)
                y_cur[X] = y_nxt[X]

            phase_idx = [0]

            def next_base():
                k = phase_idx[0]
                phase_idx[0] += 1
                return k * gate_ns * 1e-6

            def sub(base, frac):
                return tc.tile_wait_until(base + frac * gate_ns * 1e-6)

            # =========== ENCODER ===========
            n_groups = (T + GS - 1) // GS
            CH_STEPS = GS * CHUNK_GROUPS
            with tc.tile_pool(name="epsum", bufs=2, space="PSUM") as epsum:
                chunks = {}

                def load_chunk(ci):
                    s0 = ci * CH_STEPS
                    s1 = min(T, s0 + CH_STEPS)
                    xa_sb = xpool.tile([KA, CH_STEPS, B], F16, tag="xa")
                    xb_sb = xpool.tile([KB, CH_STEPS, B], F16, tag="xb")
                    if ci == 0:
                        # Per-group pieces on alternating DGE queues:
                        # group 0's x-proj matmuls only depend on the
                        # first piece, and pieces transfer in parallel.
                        engs = [nc.sync, nc.gpsimd, nc.scalar]
                        for pi, p0 in enumerate(range(0, s1 - s0, GS)):
                            p1 = min(s1 - s0, p0 + GS)
                            eng = engs[pi % len(engs)]
                            eng.dma_start(
                                out=xa_sb[:, p0:p1, :],
                                in_=xa_d.ap()[:, s0 + p0:s0 + p1, :])
                            eng.dma_start(
                                out=xb_sb[:, p0:p1, :],
                                in_=xb_d.ap()[:, s0 + p0:s0 + p1, :])
                    else:
                        nc.sync.dma_start(out=xa_sb[:, 0:s1 - s0, :],
                                          in_=xa_d.ap()[:, s0:s1, :])
                        nc.sync.dma_start(out=xb_sb[:, 0:s1 - s0, :],
                                          in_=xb_d.ap()[:, s0:s1, :])
                    chunks[ci] = (xa_sb, xb_sb)

                def make_xmm_emitters(g8, pe4):
                    steps = min(GS, T - GS * g8)
                    nc2 = steps * BC
                    ci = g8 // CHUNK_GROUPS
                    offs = (g8 % CHUNK_GROUPS) * GS
                    ems = []
                    if ENC_PRECLEAR:
                        for X in (0, 1):
                            for gi in range(4):
                                def emz(X=X, gi=gi):
                                    nc.tensor.matmul(
                                        pe4[:, X, gi, 0:nc2], zw[:],
                                        w_hh[:, 0:nc2],
                                        start=True, stop=False)
                                ems.append(emz)
                    for X in (0, 1):
                        for gi in range(4):
                            def ema(X=X, gi=gi):
                                xa_sb, _ = chunks[ci]
                                nc.tensor.matmul(
                                    pe4[:, X, gi, 0:nc2],
                                    w_iha[:, gi * GW:(gi + 1) * GW],
                                    xa_sb[:, offs:offs + steps,
                                          X * BC:(X + 1) * BC],
                                    start=not ENC_PRECLEAR, stop=False)
                            def emb(X=X, gi=gi):
                                _, xb_sb = chunks[ci]
                                nc.tensor.matmul(
                                    pe4[:, X, gi, 0:nc2],
                                    w_ihb[:, gi * GW:(gi + 1) * GW],
                                    xb_sb[:, offs:offs + steps,
                                          X * BC:(X + 1) * BC],
                                    start=False, stop=False)
                            ems += [ema, emb]
                    return ems

                load_chunk(0)
                # Decoder weights: issue after the first x chunk so they
                # don't sit ahead of it on the Sync DMA queue.
                nc.sync.dma_start(out=w_d0[:], in_=w_d0_d.ap())
                nc.sync.dma_start(out=w_dc[:], in_=w_dc_d.ap())
                nc.sync.dma_start(out=w_fc[:], in_=w_fc_d.ap())
                pe_cur = epsum.tile([128, 2, 4, GS * BC], F32, tag="pe4")
                for em in make_xmm_emitters(0, pe_cur):
                    em()
                pe_next, next_ems = None, []

                for g8 in range(n_groups):
                    steps = min(GS, T - GS * g8)
                    pci = (g8 + 2) // CHUNK_GROUPS
                    if pci * CHUNK < T * B and pci not in chunks:
                        load_chunk(pci)
                    if g8 + 1 < n_groups:
                        nci = (g8 + 1) // CHUNK_GROUPS
                        if nci not in chunks:
                            load_chunk(nci)
                        pe_next = epsum.tile([128, 2, 4, GS * BC], F32,
                                             tag="pe4")
                        next_ems = make_xmm_emitters(g8 + 1, pe_next)
                    else:
                        pe_next, next_ems = None, []
                    n_ph = 2 * steps
                    per_ph = -(-len(next_ems) // n_ph) if next_ems else 0
                    for s in range(steps):
                        t = GS * g8 + s
                        rp = t % 2
                        for X in (0, 1):
                            Y = 1 - X
                            base = next_base()
                            yt = t - 1 + X
                            with sub(base, 0.0):
                                if yt >= 0:
                                    tail(Y, (yt + 1) % 2)
                                col0 = s * BC
                                for gi in range(4):
                                    nc.tensor.matmul(
                                        pe_cur[:, X, gi, col0:col0 + BC],
                                        w_hh[:, gi * GW:(gi + 1) * GW],
                                        h2[:, X, rp * BC:(rp + 1) * BC],
                                        start=False, stop=True,
                                        skip_group_check=not (s == 0
                                                             and X == 0))
                                ph = 2 * s + X
                                for em in next_ems[ph * per_ph:
                                                   (ph + 1) * per_ph]:
                                    em()
                                tg(X, pe_cur[0:H, X, 0:4, col0:col0 + BC])
                                uv_cc(X)
                    pe_cur = pe_next
                # drain chain 1's step T-1 tail
                tail(1, T % 2)

            # =========== DECODER ===========
            with (
                tc.tile_pool(name="dpsum", bufs=3, space="PSUM") as dpsum,
                tc.tile_pool(name="fpsum", bufs=2, space="PSUM") as fpsum,
            ):
                PO = T % 2

                def emit_fc_mm(X):
                    """FC matmul for chain X (both h2 parities). Must stay
                    early: the next tail's h2 write has a WAR edge on it."""
                    pfc = fpsum.tile([2 * BC, F], F32, tag="pfc")
                    nc.tensor.matmul(pfc[:], h2[:, X, :], w_fc[:],
                                     start=True, stop=True)
                    return pfc

                def emit_fc_out(X, dp, pfc):
                    """PSUM->SBUF copy + output DMAs for steps (dp-1, dp).
                    Deferred to a late scheduling gate so the DVE COPY does
                    not head-of-line block the next tail's h2 stt."""
                    p1 = (dp + PO) % 2
                    ofc = work.tile([2 * BC, F], F32, tag=f"ofc{X}",
                                    name=f"ofc{X}")
                    nc.vector.tensor_copy(ofc[:], pfc[:])
                    nc.sync.dma_start(out=out_d.ap()[dp - 1, X],
                                      in_=ofc[p1 * BC:(p1 + 1) * BC, :])
                    nc.sync.dma_start(out=out_d.ap()[dp - 2, X],
                                      in_=ofc[(1 - p1) * BC:(2 - p1) * BC, :])

                for d in range(1, D + 1):
                    wd = w_d0 if d == 1 else w_dc
                    rp = (d - 1 + PO) % 2
                    for X in (0, 1):
                        Y = 1 - X
                        base = next_base()
                        yt = d - 1 + X
                        pfc = None
                        with sub(base, 0.0):
                            if (X == 1) or (d >= 2):
                                tail(Y, (yt + PO) % 2)
                            pd = dpsum.tile([128, 4 * BC], F32,
                                            tag=f"pd{X}", name=f"pd{X}")
                            if DEC_PRECLEAR:
                                nc.tensor.matmul(
                                    pd[:, :], zw[:], w_hh[:, 0:4 * BC],
                                    start=True, stop=False)
                            for gi in range(4):
                                nc.tensor.matmul(
                                    pd[:, gi * BC:(gi + 1) * BC],
                                    wd[:, gi * GW:(gi + 1) * GW],
                                    h2[:, X, rp * BC:(rp + 1) * BC],
                                    start=(gi == 0 and not DEC_PRECLEAR),
                                    stop=gi == 3)
                            if d >= 3 and d % 2 == 1:
                                pfc = emit_fc_mm(X)
                            tg(X, pd[0:H, :])
                            uv_cc(X)
                        if pfc is not None:
                            with sub(base, 1.25):
                                emit_fc_out(X, d - 1, pfc)
                # drain chain 1's step D tail
                tail(1, (D + PO) % 2)
                for X in (0, 1):
                    if D % 2 == 0:
                        pfc = emit_fc_mm(X)
                        emit_fc_out(X, D, pfc)
                    else:
                        pD = (D + PO) % 2
                        pfc = fpsum.tile([2 * BC, F], F32, tag="pfc")
                        nc.tensor.matmul(pfc[0:BC, :],
                                         h2[:, X, pD * BC:(pD + 1) * BC],
                                         w_fc[:], start=True, stop=True)
                        ofc = work.tile([2 * BC, F], F32, tag=f"ofc{X}",
                                        name=f"ofc{X}")
                        nc.vector.tensor_copy(ofc[0:BC, :], pfc[0:BC, :])
                        nc.sync.dma_start(out=out_d.ap()[D - 1, X],
                                          in_=ofc[0:BC, :])

    nc.compile()
    return nc


# ======================= host pre/post =======================

def _colscale():
    s = np.ones(G, np.float32)
    s[0:100] = 0.5    # i
    s[100:200] = 0.5  # f
    s[300:400] = 0.5  # o
    return s


def _pad_gates(w):
    """Chunk order [o, i, f, g] (source columns are i,f,g,o)."""
    K = w.shape[0]
    out = np.zeros((K, GP), w.dtype)
    for ci, gi in enumerate((3, 0, 1, 2)):
        out[:, ci * GW:ci * GW + 100] = w[:, gi * 100:(gi + 1) * 100]
    return out


def make_weight_arrays(enc_Wih, enc_Whh, enc_bih, enc_bhh,
                       dec_Wih, dec_Whh, dec_bih, dec_bhh, fc_W, fc_b):
    cs = _colscale()
    f64 = np.float64
    w_ih = (enc_Wih.T.astype(f64) * cs).astype(np.float32)
    w_hh = np.vstack([enc_Whh.T.astype(f64) * 0.5,
                      (enc_bih + enc_bhh).astype(f64)[None, :]]) * cs
    w_d0 = np.vstack([dec_Whh.T.astype(f64) * 0.5,
                      (dec_bih + dec_bhh).astype(f64)[None, :]]) * cs
    combo = fc_W.T.astype(f64) @ dec_Wih.T.astype(f64) + dec_Whh.T.astype(f64)
    bias_c = (fc_b.astype(f64) @ dec_Wih.T.astype(f64)
              + dec_bih.astype(f64) + dec_bhh.astype(f64))
    w_dc = np.vstack([combo * 0.5, bias_c[None, :]]) * cs
    w_fc = np.vstack([fc_W.T.astype(f64) * 0.5, fc_b.astype(f64)[None, :]])
    return {
        "w_iha": _pad_gates(w_ih[0:KA]).astype(np.float16),
        "w_ihb": _pad_gates(w_ih[KA:F]).astype(np.float16),
        "w_hh": _pad_gates(w_hh).astype(np.float16),
        "w_d0": _pad_gates(w_d0).astype(np.float16),
        "w_dc": _pad_gates(w_dc).astype(np.float16),
        "w_fc": w_fc.astype(np.float16),
    }


def make_x_arrays(x_shard, T):
    """x_shard (B, 3, T, 25, 2) -> xa (128, T, B), xb (22, T, B) fp16."""
    xt = np.ascontiguousarray(
        x_shard.transpose(1, 3, 4, 2, 0)).reshape(F, T, B)
    xt = xt.astype(np.float16)
    return {"xa": np.ascontiguousarray(xt[0:KA]),
            "xb": np.ascontiguousarray(xt[KA:F])}


def postprocess2(core_outs, T, n_cores=8):
    """core_outs: list of (D, 2, BC, F) -> full (N, 3, T, 25, 2)."""
    N = n_cores * B
    full = np.zeros((N, 3, T, 25, 2), np.float32)
    for i, o in enumerate(core_outs):
        ob = o.reshape(T - 1, B, 3, 25, 2).transpose(1, 2, 0, 3, 4)
        full[i * B:(i + 1) * B, :, 1:T] = ob
    return full


# ======================= self-contained kernel entry =======================

_NC_CACHE = {}


def _get_nc():
    if "nc" not in _NC_CACHE:
        _NC_CACHE["nc"] = build_nc2(T=T)
    return _NC_CACHE["nc"]


def kernel(x, enc_Wih, enc_Whh, enc_bih, enc_bhh,
           dec_Wih, dec_Whh, dec_bih, dec_bhh, fc_W, fc_b):
    from concourse.bass_utils import run_bass_kernel_spmd

    x = np.asarray(x, np.float32)
    nc = _get_nc()
    weights = make_weight_arrays(
        np.asarray(enc_Wih, np.float32), np.asarray(enc_Whh, np.float32),
        np.asarray(enc_bih, np.float32), np.asarray(enc_bhh, np.float32),
        np.asarray(dec_Wih, np.float32), np.asarray(dec_Whh, np.float32),
        np.asarray(dec_bih, np.float32), np.asarray(dec_bhh, np.float32),
        np.asarray(fc_W, np.float32), np.asarray(fc_b, np.float32))
    in_maps = []
    for i in range(N_CORES):
        xs = x[i * B:(i + 1) * B]
        in_maps.append({**weights, **make_x_arrays(xs, T)})

    res = run_bass_kernel_spmd(nc, in_maps, core_ids=list(range(N_CORES)))
    return postprocess2([r["out"] for r in res.results], T, N_CORES)

